# revision 21
# baseline (speedup 1.0000x reference)
"""AffNetR TRN2 kernel: out[u,i] = ((max_h cos(Z[h,u,:], X[i,:])) + 1) / 2, ^beta.

Sharding: data-parallel over users (U=8192) across 8 NeuronCores; X replicated.
Each core computes a [1024, 8192] slice of the output.

v5: bf16 everywhere, selector-matmul norm pipeline, PE-transposed reciprocals.

Inputs arrive pre-transposed and pre-cast to bf16 ([E=128, *]). Norms via
accumulating selector matmuls: chunk c's column-sum-of-squares lands on PSUM
partition c of one [16,512] (X) / [8,512] (Z) tile — no HBM bounce. Both sum
tiles are PE-transposed into one [128,96] tile so sqrt/reciprocal run
partition-parallel (~1us total instead of 8us of DVE reciprocal), then
transposed back to row layout, evacuated to bf16, and broadcast to 128
partitions with selector matmuls; a DVE multiply (PSUM operand) writes the
normalized bf16 operands. The X side folds the final /2 affine (scale=4 under
the sqrt); Z chunks normalize the lhsT operand directly.

Main loop per (u-tile 128, i-tile 512): 4 bf16 matmuls (PSUM slice order
h0,h2,h1,h3) into a [128,2048] PSUM tile (4 banks, double-buffered). The PE is
the pacing engine (~450ns per 512-row matmul, mid p-state — measured identical
for bf16/fp32r); the evacuation is kept strictly under it: ACT evacuates heads
0,2 (+0.5 bias) to bf16, DVE folds heads 1,3 with scalar_tensor_tensor
((p+0.5) max c) plus one bf16 tensor-tensor max into the bf16 output stage
(~1.7us/tile total vs the PE's ~1.8us); Sync issues the store DMAs. Output is
bf16; the host upcasts to f32.

A post-Tile pass splits excess semaphore waits onto inserted NoOps.
"""

import numpy as np

import concourse.bass as bass
import concourse.mybir as mybir
import concourse.tile as tile
from concourse.bass_utils import run_bass_kernel_spmd

F32 = mybir.dt.float32
F32R = mybir.dt.float32r
BF16 = mybir.dt.bfloat16

H = 4
U = 8192
E = 128
I = 8192
NCORES = 8
USH = U // NCORES          # 1024 users per core
UT = USH // 128            # 8 u-tiles
IT = I // 512              # 16 i-tiles
NXC = I // 512             # 16 x chunks of 512
NZC = (H * USH) // 512     # 8 z chunks of 512

_cache = {}


def _legalize_waits(nc, max_waits=1):
    """Hoist excess sem waits onto same-engine NoOps (1-wait ISA structs)."""
    cnt = 0
    for f in nc.m.functions:
        for blk in f.blocks:
            insts = blk.instructions
            out = []
            changed = False
            for inst in insts:
                si = inst.sync_info
                waits = list(si.on_wait) if si is not None and si.on_wait else []
                if len(waits) > max_waits and inst.engine is not None:
                    keep = waits[-max_waits:]
                    for w in waits[:-max_waits]:
                        nop = mybir.InstNoOp(name=f"wlg-{cnt}", ins=[], outs=[])
                        cnt += 1
                        nop.engine = inst.engine
                        nop.sync_info = mybir.SyncInfo(on_wait=[w], on_update=[])
                        out.append(nop)
                    upd = list(si.on_update) if si.on_update else []
                    inst.sync_info = mybir.SyncInfo(on_wait=keep, on_update=upd)
                    changed = True
                out.append(inst)
            if changed:
                blk.instructions = out
    return cnt


def _build():
    nc = bass.Bass()
    xt_d = nc.dram_tensor("xt", [E, I], BF16, kind="ExternalInput")
    zt_d = nc.dram_tensor("zt", [E, H * USH], BF16, kind="ExternalInput")
    # colsum stationaries: slice c is a [128, k] matrix whose column c is ones
    selx_d = nc.dram_tensor("selx", [E, NXC * NXC], BF16, kind="ExternalInput")
    selz_d = nc.dram_tensor("selz", [E, NZC * NZC], BF16, kind="ExternalInput")
    # broadcast stationaries: slice c is a [k, 128] matrix whose row c is ones
    selbx_d = nc.dram_tensor("selbx", [NXC, NXC * 128], BF16, kind="ExternalInput")
    selbz_d = nc.dram_tensor("selbz", [NZC, NZC * 128], BF16, kind="ExternalInput")
    idt_d = nc.dram_tensor("idt", [128, 128], F32R, kind="ExternalInput")
    out_d = nc.dram_tensor("out", [USH, I], BF16, kind="ExternalOutput")
    out_v = out_d[:].rearrange("(uo p) i -> p uo i", p=128)

    S = mybir.ActivationFunctionType
    A = mybir.AluOpType

    with tile.TileContext(nc) as tc:
        with tc.tile_pool(name="big", bufs=1) as big:
            xt_sb = big.tile([E, I], BF16, tag="xt_sb")
            zt_sb = big.tile([E, H * USH], BF16, tag="zt_sb")
            xtn = big.tile([E, I], BF16, tag="xtn")
            ztn = big.tile([E, H * USH], BF16, tag="ztn")
            selx = big.tile([E, NXC * NXC], BF16, tag="selx")
            selz = big.tile([E, NZC * NZC], BF16, tag="selz")
            selbx = big.tile([NXC, NXC * 128], BF16, tag="selbx")
            selbz = big.tile([NZC, NZC * 128], BF16, tag="selbz")
            idt = big.tile([128, 128], F32R, tag="idt")
            g_sb = big.tile([NZC, 512], F32R, tag="g_sb")
            gx_sb = big.tile([NXC, 512], F32R, tag="gx_sb")
            s_all = big.tile([128, 96], F32, tag="s_all")
            v_all = big.tile([128, 96], F32, tag="v_all")
            v_allr = big.tile([128, 96], F32R, tag="v_allr")
            rzb = big.tile([NZC, 512], BF16, tag="rzb")
            rxb = big.tile([NXC, 512], BF16, tag="rxb")
            half1 = big.tile([128, 1], F32, tag="half1")

            # input DMAs: z first (gates the scale path), sels on gpsimd queue
            nc.sync.dma_start(zt_sb[:, 0:2048], zt_d[:, 0:2048])
            nc.sync.dma_start(zt_sb[:, 2048:4096], zt_d[:, 2048:4096])
            nc.gpsimd.dma_start(selz, selz_d[:])
            nc.gpsimd.dma_start(selx, selx_d[:])
            nc.gpsimd.dma_start(idt, idt_d[:])
            nc.gpsimd.dma_start(selbz, selbz_d[:])
            nc.gpsimd.dma_start(selbx, selbx_d[:])
            for q, eng in enumerate((nc.sync, nc.scalar, nc.gpsimd, nc.sync)):
                s = slice(q * 2048, (q + 1) * 2048)
                eng.dma_start(xt_sb[:, s], xt_d[:, s])

            nc.vector.memset(half1, 0.5)

            sq_ctx = tc.tile_pool(name="sq", bufs=8)
            sq_pool = sq_ctx.__enter__()
            pcs_ctx = tc.tile_pool(name="pcs", bufs=1, space="PSUM")
            pcs = pcs_ctx.__enter__()

            def square(src, c, on_act):
                s = slice(c * 512, (c + 1) * 512)
                sq = sq_pool.tile([E, 512], BF16, tag="sq")
                if on_act:
                    nc.scalar.activation(sq, src[:, s], S.Square)
                else:
                    nc.vector.tensor_tensor(sq, src[:, s], src[:, s], A.mult)
                return sq

            def colsum_mm(g, sel, sq, c, npar, nchunks):
                nc.tensor.matmul(
                    g,
                    sel[:, c * npar : (c + 1) * npar],
                    sq,
                    start=(c == 0),
                    stop=(c == nchunks - 1),
                )

            # ---- column sums of squares ----
            gz = pcs.tile([NZC, 512], F32, tag="gz")
            for c in range(NZC):
                sq = square(zt_sb, c, on_act=False)
                colsum_mm(gz, selz, sq, c, NZC, NZC)
            nc.scalar.copy(g_sb, gz)
            gx = pcs.tile([NXC, 512], F32, tag="gx")
            for c in range(NXC):
                sq = square(xt_sb, c, on_act=True)
                colsum_mm(gx, selx, sq, c, NXC, NXC)
            nc.scalar.copy(gx_sb, gx)

            # ---- transpose, rsqrt partition-parallel, transpose back ----
            # z cols: col = blk*8 + c ; x cols: col = blk*16 + c
            t_all = pcs.tile([128, 96], F32R, tag="t_all")
            tza = t_all[:, 0:32]
            txa = t_all[:, 32:96]
            for blk in range(4):
                nc.tensor.transpose(
                    tza[:, blk * NZC : (blk + 1) * NZC],
                    g_sb[0:NZC, blk * 128 : (blk + 1) * 128],
                    idt[0:NZC, 0:NZC],
                )
            # z: 1/sqrt(ss)
            nc.scalar.activation(s_all[:, 0:32], tza, S.Sqrt)
            nc.vector.reciprocal(v_all[:, 0:32], s_all[:, 0:32])
            nc.scalar.copy(v_allr[:, 0:32], v_all[:, 0:32])
            tzb = pcs.tile([NZC, 512], F32R, tag="tzb")
            for blk in range(4):
                nc.tensor.transpose(
                    tzb[0:NZC, blk * 128 : (blk + 1) * 128],
                    v_allr[:, blk * NZC : (blk + 1) * NZC],
                    idt,
                )
            nc.scalar.copy(rzb, tzb)
            # z normalization (broadcast + multiply) before the x scale chain
            zprep_ctx = tc.tile_pool(name="zprep", bufs=2, space="PSUM")
            zprep = zprep_ctx.__enter__()
            for c in range(NZC):
                s = slice(c * 512, (c + 1) * 512)
                rep = zprep.tile([128, 512], F32, tag="zrep")
                nc.tensor.matmul(
                    rep,
                    selbz[0:NZC, c * 128 : (c + 1) * 128],
                    rzb,
                    start=True,
                    stop=True,
                )
                nc.vector.scalar_tensor_tensor(
                    ztn[:, s], rep, 1.0, zt_sb[:, s], op0=A.bypass, op1=A.mult
                )
            zprep_ctx.__exit__(None, None, None)
            for blk in range(4):
                nc.tensor.transpose(
                    txa[:, blk * NXC : (blk + 1) * NXC],
                    gx_sb[0:NXC, blk * 128 : (blk + 1) * 128],
                    idt[0:NXC, 0:NXC],
                )
            # x: 0.5/sqrt(ss) = 1/sqrt(4*ss)
            nc.scalar.activation(s_all[:, 32:96], txa, S.Sqrt, scale=4.0)
            nc.vector.reciprocal(v_all[:, 32:96], s_all[:, 32:96])
            nc.scalar.copy(v_allr[:, 32:96], v_all[:, 32:96])
            txb = pcs.tile([NXC, 512], F32R, tag="txb")
            for blk in range(4):
                nc.tensor.transpose(
                    txb[0:NXC, blk * 128 : (blk + 1) * 128],
                    v_allr[:, 32 + blk * NXC : 32 + (blk + 1) * NXC],
                    idt,
                )
            nc.scalar.copy(rxb, txb)

            pcs_ctx.__exit__(None, None, None)
            sq_ctx.__exit__(None, None, None)
            prep_ctx = tc.tile_pool(name="prep", bufs=4, space="PSUM")
            prep = prep_ctx.__enter__()

            # ---- broadcast scales + normalize operands ----
            def norm_chunk(src, dst, selb, scales, npar, c):
                s = slice(c * 512, (c + 1) * 512)
                rep = prep.tile([128, 512], F32, tag="rep")
                nc.tensor.matmul(
                    rep,
                    selb[0:npar, c * 128 : (c + 1) * 128],
                    scales,
                    start=True,
                    stop=True,
                )
                nc.vector.scalar_tensor_tensor(
                    dst[:, s], rep, 1.0, src[:, s], op0=A.bypass, op1=A.mult
                )

            for c in range(NXC):
                norm_chunk(xt_sb, xtn, selbx, rxb, NXC, c)

            prep_ctx.__exit__(None, None, None)

            # ---------- main loop ----------
            with (
                tc.tile_pool(name="work", bufs=4) as work,
                tc.tile_pool(name="ost", bufs=3) as ost,
                tc.tile_pool(name="pmm", bufs=2, space="PSUM") as pmm,
            ):
                # separate PSUM tiles per head (fine-grained deps/release).
                # Two tile flavors balance DVE vs ACT: normally ACT evacuates
                # heads 0,2 and DVE does two stt folds; on ~4/11 of tiles ACT
                # also evacuates head 1 so DVE's first fold is a cheap bf16
                # tensor-tensor max.
                for ut in range(UT):
                    lhs = [
                        ztn[:, h * USH + ut * 128 : h * USH + (ut + 1) * 128]
                        for h in range(H)
                    ]
                    for it in range(IT):
                        alt = (ut * IT + it) % 5 < 2
                        rhs = xtn[:, it * 512 : (it + 1) * 512]
                        ps = {}
                        for h in ((0, 1, 2, 3) if alt else (0, 2, 1, 3)):
                            p = pmm.tile([128, 512], F32, tag=f"p{h}")
                            nc.tensor.matmul(p, lhs[h], rhs, start=True, stop=True)
                            ps[h] = p
                        c0 = work.tile([128, 512], BF16, tag="c0")
                        nc.scalar.activation(
                            c0, ps[0], S.Identity, bias=half1, scale=1.0
                        )
                        c2 = work.tile([128, 512], BF16, tag="c2")
                        nc.scalar.activation(
                            c2, ps[2], S.Identity, bias=half1, scale=1.0
                        )
                        mA = work.tile([128, 512], BF16, tag="mA")
                        if alt:
                            c1 = work.tile([128, 512], BF16, tag="c1")
                            nc.scalar.activation(
                                c1, ps[1], S.Identity, bias=half1, scale=1.0
                            )
                            nc.vector.tensor_tensor(mA, c1, c0, A.max)
                        else:
                            nc.vector.scalar_tensor_tensor(
                                mA, ps[1], 0.5, c0, op0=A.add, op1=A.max
                            )
                        mB = work.tile([128, 512], BF16, tag="mB")
                        nc.vector.scalar_tensor_tensor(
                            mB, ps[3], 0.5, c2, op0=A.add, op1=A.max
                        )
                        if it % 4 == 0:
                            ostage = ost.tile([128, 2048], BF16, tag="ostage")
                        nc.vector.tensor_tensor(
                            ostage[:, (it % 4) * 512 : (it % 4 + 1) * 512],
                            mA,
                            mB,
                            A.max,
                        )
                        if ut == UT - 1 and it >= 12:
                            j = it % 4
                            nc.sync.dma_start(
                                out_v[:, ut, (12 + j) * 512 : (13 + j) * 512],
                                ostage[:, j * 512 : (j + 1) * 512],
                            )
                        elif it % 4 == 3:
                            ig = it // 4
                            nc.sync.dma_start(
                                out_v[:, ut, ig * 2048 : (ig + 1) * 2048],
                                ostage,
                            )

    _legalize_waits(nc)
    return nc


def _sel_hosts():
    import ml_dtypes

    bf = ml_dtypes.bfloat16
    selx = np.zeros((E, NXC * NXC), dtype=bf)
    for c in range(NXC):
        selx[:, c * NXC + c] = 1.0
    selz = np.zeros((E, NZC * NZC), dtype=bf)
    for c in range(NZC):
        selz[:, c * NZC + c] = 1.0
    selbx = np.zeros((NXC, NXC * 128), dtype=bf)
    for c in range(NXC):
        selbx[c, c * 128 : (c + 1) * 128] = 1.0
    selbz = np.zeros((NZC, NZC * 128), dtype=bf)
    for c in range(NZC):
        selbz[c, c * 128 : (c + 1) * 128] = 1.0
    idt = np.eye(128, dtype=np.float32)
    return selx, selz, selbx, selbz, idt


def _in_maps(X, Z):
    import ml_dtypes

    bf = ml_dtypes.bfloat16
    X = np.asarray(X, dtype=np.float32)
    Z = np.asarray(Z, dtype=np.float32)
    xt = np.ascontiguousarray(X.T).astype(bf)            # [128, 8192]
    selx, selz, selbx, selbz, idt = _sel_hosts()
    in_maps = []
    for c in range(NCORES):
        zs = Z[:, c * USH : (c + 1) * USH, :]            # [4, 1024, 128]
        zt = np.ascontiguousarray(
            zs.transpose(2, 0, 1).reshape(E, H * USH)
        ).astype(bf)                                     # [128, 4096]
        in_maps.append(
            {
                "xt": xt,
                "zt": zt,
                "selx": selx,
                "selz": selz,
                "selbx": selbx,
                "selbz": selbz,
                "idt": idt,
            }
        )
    return in_maps


def kernel(X, Z, beta):
    in_maps = _in_maps(X, Z)
    if "nc" not in _cache:
        _cache["nc"] = _build()
    res = run_bass_kernel_spmd(_cache["nc"], in_maps, list(range(NCORES))).results
    out = np.concatenate([r["out"] for r in res], axis=0).astype(np.float32)

    b = float(np.asarray(beta))
    if b != 1.0:
        out = np.power(out, b).astype(np.float32)
    return out
